# revision 17
# baseline (speedup 1.0000x reference)
"""Trainium2 Bass kernel for nn_CausalSelfAttention_16810501996824.

Head-sharded (tensor-parallel) causal self-attention over 8 NeuronCores:
each core owns 2 of the 16 heads end-to-end (QKV projection, RMS norm,
rotary, causal attention with sigmoid gate and lambda-blended V), then an
AllGather of the per-head context vectors and an output-column-sharded
c_proj. Host only reshapes/concats shards.

Self-contained: hardcodes all shapes; builds + compiles the Bass module on
first call and caches the jitted SPMD executable.
"""
import json

import numpy as np

# ---------------------------------------------------------------------------
# Problem constants
# ---------------------------------------------------------------------------
DIM = 1024
N_HEAD = 16
T = 2048
HD = 64                 # head dim
GATE_IN = 12
ROPE_BASE = 10000.0
ATTN_SCALE = 0.1
EPS = 1e-6
N_CORES = 8
HPC = N_HEAD // N_CORES  # heads per core = 2
C = HPC * HD             # channels per core = 128
NT512 = T // 512         # 4 t-windows
NS128 = T // 128         # 16 s-blocks

# ---------------------------------------------------------------------------
# Workaround: the staged walrus build allows at most 1 sem wait per
# instruction (2 for EventSemaphore); stock Tile piles multiple waits onto
# one instruction. Split extras onto single-wait NoOps at serialization.
# ---------------------------------------------------------------------------
_WAIT_CAP = {"EventSemaphore": 2}


def _split_multi_waits(bir: dict) -> dict:
    for fn in bir.get("functions", []):
        for blk in fn.get("blocks", []):
            out = []
            changed = False
            for inst in blk.get("instructions", []):
                si = inst.get("sync_info") or {}
                waits = si.get("on_wait") or []
                cap = _WAIT_CAP.get(inst.get("opcode"), 1)
                if len(waits) > cap:
                    changed = True
                    for j, w in enumerate(waits[cap:]):
                        out.append({
                            "debug": inst.get("debug", 0),
                            "engine": inst["engine"],
                            "ins": [], "outs": [],
                            "name": f"{inst['name']}-wsplit{j}",
                            "opcode": "NoOp",
                            "sync_info": {"on_update": [], "on_wait": [w]},
                            "text_hint": "wait_split",
                        })
                    si = dict(si)
                    si["on_wait"] = waits[:cap]
                    inst = dict(inst)
                    inst["sync_info"] = si
                out.append(inst)
            if changed:
                blk["instructions"] = out
    return bir


def _install_patches():
    import concourse.bass as bass
    if getattr(bass.Bass, "_wait_split_patched", False):
        return
    orig = bass.Bass.to_json_bytes

    def patched(self, *a, **k):
        return json.dumps(_split_multi_waits(json.loads(orig(self, *a, **k)))).encode()

    bass.Bass.to_json_bytes = patched
    bass.Bass._wait_split_patched = True


# ---------------------------------------------------------------------------
# Bass module
# ---------------------------------------------------------------------------

def _build_module(repeat=1, phases=4):
    import concourse.bass as bass
    import concourse.mybir as mybir
    import concourse.tile as tile

    F32 = mybir.dt.float32
    F32R = mybir.dt.float32r
    AF = mybir.ActivationFunctionType

    nc = bass.Bass()

    xT = nc.declare_dram_parameter("xT", [DIM, T], F32R, isOutput=False)
    wqkvT = nc.declare_dram_parameter("wqkvT", [DIM, 3 * C], F32R, isOutput=False)
    wgT = nc.declare_dram_parameter("wgT", [GATE_IN, HPC], F32R, isOutput=False)
    wprojT = nc.declare_dram_parameter("wprojT", [DIM, C], F32R, isOutput=False)
    v1lam = nc.declare_dram_parameter("v1lam", [T, C], F32R, isOutput=False)
    lam1 = nc.declare_dram_parameter("lam1", [128, 1], F32, isOutput=False)
    cosd = nc.declare_dram_parameter("cosd", [C, T], F32, isOutput=False)
    sind = nc.declare_dram_parameter("sind", [C, T], F32, isOutput=False)
    swapm = nc.declare_dram_parameter("swapm", [128, 128], F32R, isOutput=False)
    identm = nc.declare_dram_parameter("identm", [128, 128], F32R, isOutput=False)
    outT = nc.declare_dram_parameter("outT", [C, T], F32, isOutput=True)

    HT = T // 2
    y_loc = [nc.dram_tensor(f"y_loc{i}", [C, HT], F32R) for i in range(2)]
    y_full = [nc.dram_tensor(f"y_full{i}", [N_CORES * C, HT], F32R, addr_space="Shared")
              for i in range(2)]

    with nc.allow_low_precision(reason="f32r matmul pipeline"), \
            tile.TileContext(nc) as tc:
      for _rep in range(repeat):
        with tc.tile_pool(name=f"persist{_rep}", bufs=1) as persist, \
             tc.tile_pool(name=f"vaug{_rep}", bufs=1) as vaug_pool:
            # ---- persistent tiles ----
            qt = persist.tile([128, T], F32R)      # qT, both heads stacked
            kt = persist.tile([128, T], F32R)
            vt = persist.tile([128, T], F32R)      # vT (pre-blend)
            gtmp = persist.tile([HPC, T], F32R)
            ident = persist.tile([128, 128], F32R)
            swp = persist.tile([128, 128], F32R)
            ones_col = persist.tile([65, 64], F32R)
            mlo = persist.tile([1, 128], F32R)
            mhi = persist.tile([1, 128], F32R)
            blo = persist.tile([128, 1], F32R)
            bhi = persist.tile([128, 1], F32R)
            lam1_sb = persist.tile([128, 1], F32)
            eps_sb = persist.tile([128, 1], F32)
            v_aug = [[vaug_pool.tile([128, HD + 1], F32R, name=f"va{h}_{si}", tag=f"va{h}_{si}")
                      for si in range(NS128)] for h in range(HPC)]

            nc.sync.dma_start(out=swp, in_=swapm[:])
            nc.sync.dma_start(out=lam1_sb, in_=lam1[:])
            nc.vector.memset(eps_sb, EPS)
            nc.sync.dma_start(out=ident, in_=identm[:])
            nc.vector.memset(ones_col.bitcast(F32), 1.0)
            nc.vector.memset(mlo.bitcast(F32), 0.0)
            nc.vector.memset(mlo.bitcast(F32)[:, 0:64], 1.0)
            nc.vector.memset(mhi.bitcast(F32), 0.0)
            nc.vector.memset(mhi.bitcast(F32)[:, 64:128], 1.0)
            nc.vector.memset(blo.bitcast(F32), 0.0)
            nc.vector.memset(blo.bitcast(F32)[0:64, :], 1.0 / HD)
            nc.vector.memset(bhi.bitcast(F32), 0.0)
            nc.vector.memset(bhi.bitcast(F32)[64:128, :], 1.0 / HD)

            # =============================================================
            # Phase 1: QKV projections + RMS norm + rotary + gate
            # =============================================================
            with tc.tile_pool(name="p1sbuf", bufs=1) as p1, \
                 tc.tile_pool(name="p1temp", bufs=2) as p1t, \
                 tc.tile_pool(name="p1acc", bufs=1, space="PSUM") as p1acc, \
                 tc.tile_pool(name="p1aux", bufs=2, space="PSUM") as p1aux, \
                 tc.tile_pool(name="p1small", bufs=3, space="PSUM") as p1s:
                xts = [p1.tile([128, T], F32R, name=f"xt{d}", tag=f"xt{d}") for d in range(8)]
                wts = [p1.tile([128, 3 * C], F32R, name=f"wt{d}", tag=f"wt{d}") for d in range(8)]
                wg_sb = p1.tile([GATE_IN, HPC], F32R)
                for d in range(8):
                    nc.gpsimd.dma_start(out=xts[d], in_=xT[128 * d:128 * (d + 1), :])
                    nc.gpsimd.dma_start(out=wts[d], in_=wqkvT[128 * d:128 * (d + 1), :])
                nc.sync.dma_start(out=wg_sb, in_=wgT[:])

                for tj in range(NT512):
                    ts = slice(512 * tj, 512 * (tj + 1))
                    cos_sb = p1t.tile([C, 512], F32, tag="cos")
                    sin_sb = p1t.tile([C, 512], F32, tag="sin")
                    nc.gpsimd.dma_start(out=cos_sb, in_=cosd[:, ts])
                    nc.gpsimd.dma_start(out=sin_sb, in_=sind[:, ts])

                    q_ps = p1acc.tile([128, 512], F32, tag="q_ps")
                    k_ps = p1acc.tile([128, 512], F32, tag="k_ps")
                    v_ps = p1acc.tile([128, 512], F32, tag="v_ps")
                    for d in range(8):
                        nc.tensor.matmul(q_ps, wts[d][:, 0:128], xts[d][:, ts],
                                         start=(d == 0), stop=(d == 7))
                    for d in range(8):
                        nc.tensor.matmul(k_ps, wts[d][:, 128:256], xts[d][:, ts],
                                         start=(d == 0), stop=(d == 7))
                    for d in range(8):
                        nc.tensor.matmul(v_ps, wts[d][:, 256:384], xts[d][:, ts],
                                         start=(d == 0), stop=(d == 7))
                    nc.scalar.copy(vt[:, ts], v_ps)

                    # gate: one M=2 matmul (K=12), sigmoid into gtmp rows 0,1
                    g_ps = p1s.tile([HPC, 512], F32, tag="sm")
                    nc.tensor.matmul(g_ps, wg_sb, xts[0][0:GATE_IN, ts],
                                     start=True, stop=True)
                    nc.scalar.activation(gtmp[:, ts], g_ps, AF.Sigmoid)

                    for name, r_ps, dst in (("q", q_ps, qt), ("k", k_ps, kt)):
                        raw = p1t.tile([128, 512], F32R, tag=f"{name}raw")
                        nc.scalar.copy(raw, r_ps)
                        sq = p1t.tile([128, 512], F32R, tag=f"{name}sq")
                        nc.scalar.activation(sq, r_ps, AF.Square)
                        ms0 = p1s.tile([1, 512], F32, tag="sm")
                        ms1 = p1s.tile([1, 512], F32, tag="sm")
                        nc.tensor.matmul(ms0, blo, sq, start=True, stop=True)
                        nc.tensor.matmul(ms1, bhi, sq, start=True, stop=True)
                        rt0 = p1t.tile([1, 512], F32R, tag=f"{name}rt0")
                        rt1 = p1t.tile([1, 512], F32R, tag=f"{name}rt1")
                        nc.scalar.activation(rt0, ms0, AF.Sqrt, bias=eps_sb[0:1, :])
                        nc.scalar.activation(rt1, ms1, AF.Sqrt, bias=eps_sb[0:1, :])
                        nc.vector.reciprocal(rt0, rt0)
                        nc.vector.reciprocal(rt1, rt1)
                        bc_ps = p1aux.tile([128, 512], F32, tag="aux")
                        nc.tensor.matmul(bc_ps, mlo, rt0, start=True, stop=False)
                        nc.tensor.matmul(bc_ps, mhi, rt1, start=False, stop=True)
                        sw_ps = p1aux.tile([128, 512], F32, tag="aux")
                        nc.tensor.matmul(sw_ps, swp, raw, start=True, stop=True)
                        t1 = p1t.tile([128, 512], F32, tag=f"{name}t1")
                        nc.vector.tensor_mul(t1, raw.bitcast(F32), cos_sb)
                        t2 = p1t.tile([128, 512], F32, tag=f"{name}t2")
                        nc.vector.tensor_mul(t2, sw_ps, sin_sb)
                        nc.vector.tensor_add(t1, t1, t2)
                        nc.vector.tensor_mul(dst[:, ts], t1.bitcast(F32R),
                                             bc_ps.bitcast(F32R))

                # vT -> v_nat blocks (+ lambda blend + ones col)
                for si in range(NS128):
                    ss = slice(128 * si, 128 * (si + 1))
                    tr_ps = p1aux.tile([128, 128], F32R, tag="aux")
                    nc.tensor.transpose(tr_ps, vt[:, ss], ident)
                    vl = p1t.tile([128, C], F32R, tag="vl")
                    nc.gpsimd.dma_start(out=vl, in_=v1lam[ss, :])
                    for h in range(HPC):
                        va = v_aug[h][si]
                        nc.scalar.activation(va[:, 0:HD], tr_ps[:, HD * h:HD * (h + 1)],
                                             AF.Copy, scale=lam1_sb)
                        nc.vector.tensor_add(va[:, 0:HD], va[:, 0:HD],
                                             vl[:, HD * h:HD * (h + 1)])
                        nc.vector.memset(va.bitcast(F32)[:, HD:HD + 1], 1.0)

            # =============================================================
            # Phase 2: causal attention per head
            # =============================================================
            if phases < 2:
                nc.sync.dma_start(out=outT[:].bitcast(F32R), in_=qt)
                continue
            with tc.tile_pool(name="p2t", bufs=3) as p2t, \
                 tc.tile_pool(name="p2small", bufs=2) as p2s, \
                 tc.tile_pool(name="masks", bufs=1) as mask_pool, \
                 tc.tile_pool(name="sps", bufs=3, space="PSUM") as sps_pool, \
                 tc.tile_pool(name="yps", bufs=2, space="PSUM") as yps_pool, \
                 tc.tile_pool(name="bcps", bufs=2, space="PSUM") as bcps_pool:
                # causal 0/1 masks for the 4 diagonal-band offsets, built once
                dmask = [mask_pool.tile([128, 512], F32R, name=f"dm{k}", tag=f"dm{k}")
                         for k in range(4)]
                for k in range(4):
                    nc.vector.memset(dmask[k].bitcast(F32), 1.0)
                    nc.gpsimd.affine_select(
                        out=dmask[k], in_=dmask[k],
                        compare_op=mybir.AluOpType.is_ge,
                        fill=0.0, base=-(128 * k),
                        channel_multiplier=-1, pattern=[[1, 512]])
                for tj in range(NT512):
                    ts = slice(512 * tj, 512 * (tj + 1))
                    hts = slice(512 * (tj % 2), 512 * (tj % 2) + 512)
                    for h in range(HPC):
                        hs = slice(HD * h, HD * (h + 1))
                        nsb = 4 * tj + 4
                        g64 = p2s.tile([65, 512], F32R, tag="g64")
                        nc.sync.dma_start(out=g64[64:65, :], in_=gtmp[h:h + 1, ts])
                        y_ps = yps_pool.tile([65, 512], F32, tag="y")
                        pending = None
                        for si in range(nsb):
                            ss = slice(128 * si, 128 * (si + 1))
                            s_ps = sps_pool.tile([128, 512], F32, tag="s")
                            nc.tensor.matmul(s_ps, kt[hs, ss], qt[hs, ts],
                                             start=True, stop=True)
                            p_sb = p2t.tile([128, 512], F32R, tag="p", bufs=4)
                            nc.scalar.activation(p_sb, s_ps, AF.Exp, scale=ATTN_SCALE)
                            ko = si - 4 * tj
                            if ko >= 0:
                                nc.vector.tensor_mul(p_sb, p_sb, dmask[ko])
                            if pending is not None:
                                psi, pp = pending
                                nc.tensor.matmul(y_ps, v_aug[h][psi], pp,
                                                 start=(psi == 0), stop=False)
                            pending = (si, p_sb)
                        psi, pp = pending
                        nc.tensor.matmul(y_ps, v_aug[h][psi], pp,
                                         start=(psi == 0), stop=True)
                        u = p2s.tile([65, 512], F32R, tag="u")
                        nc.vector.reciprocal(u[64:65, :], y_ps[64:65, :])
                        cs_row = p2s.tile([65, 512], F32R, tag="cs")
                        nc.vector.tensor_mul(cs_row[64:65, :], u[64:65, :],
                                             g64[64:65, :])
                        bc_ps = bcps_pool.tile([64, 512], F32, tag="bc")
                        nc.tensor.matmul(bc_ps, ones_col[64:65, :], cs_row[64:65, :],
                                         start=True, stop=True)
                        cs_sb = p2s.tile([64, 512], F32, tag="csb")
                        nc.scalar.copy(cs_sb, bc_ps)
                        yft = p2t.tile([64, 512], F32R, tag="yft")
                        nc.vector.tensor_mul(yft, y_ps[0:64, :], cs_sb)
                        nc.gpsimd.dma_start(out=y_loc[tj // 2][64 * h:64 * (h + 1), hts],
                                            in_=yft)
                    if tj == 1 and phases >= 3:
                        nc.gpsimd.collective_compute(
                            "AllGather", mybir.AluOpType.bypass,
                            ins=[y_loc[0][:]], outs=[y_full[0][:]],
                            replica_groups=[list(range(N_CORES))],
                        )

            # =============================================================
            # Phase 3: AllGather (second half; first half fired inside P2)
            # =============================================================
            if phases < 3:
                nc.sync.dma_start(out=outT[:, 0:HT].bitcast(F32R), in_=y_loc[0][:])
                continue
            nc.gpsimd.collective_compute(
                "AllGather", mybir.AluOpType.bypass,
                ins=[y_loc[1][:]], outs=[y_full[1][:]],
                replica_groups=[list(range(N_CORES))],
            )

            # =============================================================
            # Phase 4: output projection (e-slice of out^T)
            # =============================================================
            if phases < 4:
                nc.sync.dma_start(out=outT[:, 0:HT].bitcast(F32R),
                                  in_=y_full[0][0:C, :])
                continue
            with tc.tile_pool(name="p4", bufs=3) as p4, \
                 tc.tile_pool(name="p4o", bufs=1) as p4o, \
                 tc.tile_pool(name="p4w", bufs=1) as p4w, \
                 tc.tile_pool(name="ops", bufs=1, space="PSUM") as ops_pool:
                wp = [p4w.tile([128, C], F32R, name=f"wp{cc}", tag=f"wp{cc}") for cc in range(8)]
                for cc in range(8):
                    nc.sync.dma_start(out=wp[cc], in_=wprojT[128 * cc:128 * (cc + 1), :])
                o_ps = [ops_pool.tile([128, 512], F32, name=f"o{tj}", tag=f"o{tj}")
                        for tj in range(NT512)]
                yfc = p4.tile([128, 8, T], F32R, tag="yfc", bufs=1)
                yfv = [y_full[i].rearrange("(cc p) t -> p cc t", p=128) for i in range(2)]
                for cc in range(8):
                    nc.gpsimd.dma_start(out=yfc[:, cc, 0:HT], in_=yfv[0][:, cc, :])
                    nc.gpsimd.dma_start(out=yfc[:, cc, HT:T], in_=yfv[1][:, cc, :])
                for cc in range(8):
                    for tj in range(NT512):
                        nc.tensor.matmul(o_ps[tj], wp[cc], yfc[:, cc, 512 * tj:512 * (tj + 1)],
                                         start=(cc == 0), stop=(cc == 7))
                o_sb = p4o.tile([128, T], F32, tag="osb")
                for tj in range(NT512):
                    nc.scalar.copy(o_sb[:, 512 * tj:512 * (tj + 1)], o_ps[tj])
                nc.gpsimd.dma_start(out=outT[:], in_=o_sb)

    return nc


# ---------------------------------------------------------------------------
# Host-side prep + cached runner
# ---------------------------------------------------------------------------

def _rotary_tables():
    i = np.arange(0, HD, 2, dtype=np.float32)
    inv_freq = (np.float32(1.0) / np.power(np.float32(ROPE_BASE),
                                           i / np.float32(HD))).astype(np.float32)
    t = np.arange(T, dtype=np.float32)
    freqs = t[:, None] * inv_freq[None, :]          # [T, 32]
    cos = np.cos(freqs).astype(np.float32)
    sin = np.sin(freqs).astype(np.float32)
    half = HD // 2
    cosd = np.empty((C, T), np.float32)
    sind = np.empty((C, T), np.float32)
    for h in range(HPC):
        base = HD * h
        cosd[base:base + half] = cos.T
        cosd[base + half:base + HD] = cos.T
        sind[base:base + half] = sin.T
        sind[base + half:base + HD] = -sin.T
    return cosd, sind


def _swap_matrix():
    m = np.zeros((128, 128), np.float32)
    half = HD // 2
    for r in range(128):
        blk, off = divmod(r, HD)
        src = blk * HD + ((off + half) % HD)
        m[src, r] = 1.0
    return m


_CACHE = {}


def _get_runner(repeat=1, phases=4):
    key = f"runner{repeat}_{phases}"
    if key in _CACHE:
        return _CACHE[key]
    _install_patches()
    nc = _build_module(repeat, phases)

    import jax
    import concourse.mybir as mybir
    from jax.sharding import Mesh, PartitionSpec
    from jax.experimental.shard_map import shard_map
    from concourse import bass2jax

    bass2jax.install_neuronx_cc_hook()
    partition_name = nc.partition_id_tensor.name if nc.partition_id_tensor else None
    in_names, out_names, out_avals, zero_outs = [], [], [], []
    for alloc in nc.m.functions[0].allocations:
        if not isinstance(alloc, mybir.MemoryLocationSet):
            continue
        name = alloc.memorylocations[0].name
        if alloc.kind == "ExternalInput":
            if name != partition_name:
                in_names.append(name)
        elif alloc.kind == "ExternalOutput":
            shape = tuple(alloc.tensor_shape)
            dtype = mybir.dt.np(alloc.dtype)
            out_names.append(name)
            out_avals.append(jax.core.ShapedArray(shape, dtype))
            zero_outs.append(np.zeros(shape, dtype))
    all_in_names = in_names + out_names
    if partition_name is not None:
        all_in_names.append(partition_name)
    n_params, n_outs = len(in_names), len(out_avals)

    def _body(*args):
        operands = list(args)
        if partition_name is not None:
            operands.append(bass2jax.partition_id_tensor())
        return tuple(bass2jax._bass_exec_p.bind(
            *operands,
            out_avals=tuple(out_avals),
            in_names=tuple(all_in_names),
            out_names=tuple(out_names),
            lowering_input_output_aliases=(),
            sim_require_finite=True, sim_require_nnan=True, nc=nc,
        ))

    devices = jax.devices()[:N_CORES]
    mesh = Mesh(np.asarray(devices), ("core",))
    fn = jax.jit(
        shard_map(_body, mesh=mesh,
                  in_specs=(PartitionSpec("core"),) * (n_params + n_outs),
                  out_specs=(PartitionSpec("core"),) * n_outs,
                  check_rep=False),
        keep_unused=True,
    )
    state = {
        "fn": fn, "in_names": in_names, "out_names": out_names,
        "out_avals": out_avals, "zero_outs": zero_outs, "nc": nc,
    }
    _CACHE[key] = state
    return state


def _prep_inputs(x, v1, Wq, Wk, Wv, Wproj, lamb, Wgate):
    x = np.asarray(x, np.float32)
    v1 = np.asarray(v1, np.float32)
    lam = np.float32(np.asarray(lamb))
    xT = np.ascontiguousarray(x[0].T)
    cosd, sind = _rotary_tables()
    swapm = _swap_matrix()
    lam1 = np.full((128, 1), np.float32(1.0) - lam, np.float32)
    in_maps = []
    for r in range(N_CORES):
        rows = slice(C * r, C * (r + 1))
        heads = slice(HPC * r, HPC * (r + 1))
        wqkvT = np.ascontiguousarray(
            np.concatenate([np.asarray(Wq)[rows].T, np.asarray(Wk)[rows].T,
                            np.asarray(Wv)[rows].T], axis=1).astype(np.float32))
        in_maps.append({
            "xT": xT,
            "wqkvT": wqkvT,
            "wgT": np.ascontiguousarray(np.asarray(Wgate)[heads].T.astype(np.float32)),
            "wprojT": np.ascontiguousarray(np.asarray(Wproj)[rows].T.astype(np.float32)),
            "v1lam": np.ascontiguousarray((lam * v1[0][:, rows]).astype(np.float32)),
            "lam1": lam1,
            "cosd": cosd,
            "sind": sind,
            "swapm": swapm,
            "identm": np.eye(128, dtype=np.float32),
        })
    return in_maps


def _run(in_maps):
    st = _get_runner()
    concat_in = [
        np.ascontiguousarray(np.concatenate([in_maps[c][n] for c in range(N_CORES)],
                                            axis=0))
        for n in st["in_names"]
    ]
    concat_zeros = [
        np.zeros((N_CORES * z.shape[0], *z.shape[1:]), z.dtype)
        for z in st["zero_outs"]
    ]
    outs = st["fn"](*concat_in, *concat_zeros)
    outs = [np.asarray(o) for o in outs]
    return {n: outs[i].reshape(N_CORES, *st["out_avals"][i].shape)
            for i, n in enumerate(st["out_names"])}


def kernel(x, v1, Wq, Wk, Wv, Wproj, lamb, Wgate):
    in_maps = _prep_inputs(x, v1, Wq, Wk, Wv, Wproj, lamb, Wgate)
    res = _run(in_maps)
    outT = res["outT"]                                     # [cores, C, T]
    y = np.empty((1, T, DIM), np.float32)
    for r in range(N_CORES):
        y[0, :, C * r:C * (r + 1)] = outT[r].T
    return y, np.asarray(v1, np.float32)
